# revision 10
# baseline (speedup 1.0000x reference)
"""Trainium2 Bass kernel for nn_MetricNet (512-step elementwise Euler recurrence).

Strategy: pure data parallel over the batch axis — each of the 8 NeuronCores
gets 16384 frequencies laid out as a [128 partitions x 128 free] f32 tile that
lives in SBUF for the whole 512-step recurrence.

Per-step math is algebraically reduced to exactly 8 fused DVE
scalar_tensor_tensor instructions by:
  - scaling both state variables by m = 2*dz*omega (per element), which turns
    every per-element coefficient in the update into either a global scalar or
    one of two fixed per-element tensors (m, W = m^2/2),
  - shifting the Re-state by inv1 (u = Re + inv1), which collapses the
    completing-the-square cross terms,
  - tracking a global additive offset beta on the Im-state on the host, which
    absorbs the per-step scalar source term for free.

    X_j = m*(Re_j + inv1_j),  YS_j = m*Im_j = Yh_j + beta_j
    T1 = (Yh + (c1+beta)) * X          Xv = m*kt + T1      (= m * Re_next_shifted)
    Xn = m*(kt+delta) + T1             A  = (Yh+beta) - Xv
    B  = (Yh+beta) + Xv                C2 = (A*0.5) * B    (= (YS^2 - Xv^2)/2)
    E  = W*S + C2                      Yh' = Yh*c1 + E     (beta' = c1*beta + sigma)

All per-step scalars are host-precomputed in float64 from B and PiT and baked
as fp32 immediates.
"""

import numpy as np

import concourse.bass as bass
import concourse.mybir as mybir
import bass_rust as _br
from concourse import tile
from concourse.bass_utils import run_bass_kernel_spmd

# walrus's codegen rejects instructions carrying more than ~2 sync-wait
# commands, but Tile's exit path hangs the full end-of-kernel wait set
# (one per engine/DMA lane used) on a single SP drain. Split those waits
# across dedicated one-wait NOPs ahead of a bare drain instead.
_orig_drain_and_barrier = tile.TileContext._drain_and_barrier


def _split_drain_and_barrier(self, tick_clock, wait_clock):
    nc = self.nc
    probe = nc.sync.nop()
    wait_clock.add_sem_waits(
        probe.ins, _br.ScopedClock({None: tick_clock.global_clock})
    )
    si = probe.ins.sync_info
    if si is not None and len(si.on_wait) > 1:
        waits = list(si.on_wait)
        probe.ins.sync_info = _br.SyncInfo(
            on_wait=waits[:1], on_update=list(si.on_update)
        )
        for w in waits[1:]:
            extra = nc.sync.nop()
            extra.ins.sync_info = _br.SyncInfo(on_wait=[w], on_update=[])
    nc.sync.drain()
    nc.all_engine_barrier()
    popped = nc._tile_sem_poison_stack.pop()
    assert popped is self._sem_poison
    nc.clear_and_free_semaphores(list(self.sems.allocated().values()))
    nc.all_engine_barrier()


tile.TileContext._drain_and_barrier = _split_drain_and_barrier


N_LAYERS = 512
Z_INI = 0.0
DEL_Z = 0.9 / 512.0
MU = 1.0
BATCH = 131072
N_CORES = 8
P = 128
F = BATCH // N_CORES // P  # 128

F32 = mybir.dt.float32
ALU = mybir.AluOpType


def _host_scalars(B: np.ndarray, p: float):
    """Per-step scalar schedule, float64."""
    zs = Z_INI + DEL_Z * np.arange(N_LAYERS, dtype=np.float64)
    b1 = B.astype(np.float64)[:N_LAYERS]
    b2 = B.astype(np.float64)[1 : N_LAYERS + 1]
    g = 1.0 - b2 / b1
    c1 = 1.0 + g
    inv1 = 1.0 / (p * (1.0 - zs))
    inv2 = inv1 / (1.0 - zs)
    kt = -DEL_Z * inv2
    delta = np.empty(N_LAYERS)
    delta[:-1] = inv1[1:] - inv1[:-1]
    delta[-1] = -inv1[-1]  # so that final Xn = m * Re_final
    S = -inv2 / p + inv1**2 + 1.0 / b1**2
    T = DEL_Z * zs**2 * (MU * MU) / b1
    sigma = -2.0 * DEL_Z * T
    # beta_j tracked through the recurrence: beta' = c1*beta + sigma
    beta = np.zeros(N_LAYERS + 1)
    for j in range(N_LAYERS):
        beta[j + 1] = c1[j] * beta[j] + sigma[j]
    return c1, kt, delta, S, beta, inv1


def _build_bass(c1, kt, delta, S, beta, inv1_0):
    nc = bass.Bass()
    # packed input: [re | im | om] along the free axis; packed output: [re | im]
    x_in = nc.dram_tensor("x_in", [P, 3 * F], F32, kind="ExternalInput")
    x_out = nc.dram_tensor("x_out", [P, 2 * F], F32, kind="ExternalOutput")

    f = float  # immediates
    with tile.TileContext(nc) as tc:
        with tc.tile_pool(name="pool", bufs=1) as pool:
            xin = pool.tile([P, 3 * F], F32)
            nc.gpsimd.dma_start(xin[:], x_in[:])
            re = xin[:, 0:F]
            im = xin[:, F : 2 * F]
            om = xin[:, 2 * F : 3 * F]

            m = pool.tile([P, F], F32)
            W = pool.tile([P, F], F32)
            Xa = pool.tile([P, F], F32)
            Xb = pool.tile([P, F], F32)
            Ya = pool.tile([P, F], F32)
            Yb = pool.tile([P, F], F32)
            T1 = pool.tile([P, F], F32)
            Xv = pool.tile([P, F], F32)
            A = pool.tile([P, F], F32)
            Bt = pool.tile([P, F], F32)
            C2 = pool.tile([P, F], F32)
            E = pool.tile([P, F], F32)
            minv = pool.tile([P, F], F32)
            xout = pool.tile([P, 2 * F], F32)
            reo = xout[:, 0:F]
            imo = xout[:, F : 2 * F]

            v = nc.vector
            stt = v.scalar_tensor_tensor
            # m = 2*dz*omega ; W = m*m/2
            v.tensor_scalar_mul(m[:], om, f(2.0 * DEL_Z))
            stt(W[:], m[:], 0.5, m[:], ALU.mult, ALU.mult)
            # X0 = (re + inv1_0) * m ; Y0 = im * m
            stt(Xa[:], re, f(inv1_0), m[:], ALU.add, ALU.mult)
            v.tensor_mul(Ya[:], im, m[:])

            X, Xn = Xa, Xb
            Y, Yn = Ya, Yb
            for j in range(N_LAYERS):
                stt(T1[:], Y[:], f(c1[j] + beta[j]), X[:], ALU.add, ALU.mult)
                stt(Xv[:], m[:], f(kt[j]), T1[:], ALU.mult, ALU.add)
                stt(Xn[:], m[:], f(kt[j] + delta[j]), T1[:], ALU.mult, ALU.add)
                stt(A[:], Y[:], f(beta[j]), Xv[:], ALU.add, ALU.subtract)
                stt(Bt[:], Y[:], f(beta[j]), Xv[:], ALU.add, ALU.add)
                stt(C2[:], A[:], 0.5, Bt[:], ALU.mult, ALU.mult)
                stt(E[:], W[:], f(S[j]), C2[:], ALU.mult, ALU.add)
                stt(Yn[:], Y[:], f(c1[j]), E[:], ALU.mult, ALU.add)
                X, Xn = Xn, X
                Y, Yn = Yn, Y

            v.reciprocal(minv[:], m[:])
            v.tensor_mul(reo, X[:], minv[:])
            stt(imo, Y[:], f(beta[N_LAYERS]), minv[:], ALU.add, ALU.mult)
            nc.sync.dma_start(x_out[:], xout[:])
    return nc


def kernel(Re_s, Im_s, omega, PiT, B, _trace=False):
    Re_s = np.ascontiguousarray(Re_s, dtype=np.float32)
    Im_s = np.ascontiguousarray(Im_s, dtype=np.float32)
    omega = np.ascontiguousarray(omega, dtype=np.float32)
    p = float(np.asarray(PiT).reshape(-1)[0])
    c1, kt, delta, S, beta, inv1 = _host_scalars(np.asarray(B), p)

    nc = _build_bass(c1, kt, delta, S, beta, float(inv1[0]))

    re8 = Re_s.reshape(N_CORES, P, F)
    im8 = Im_s.reshape(N_CORES, P, F)
    om8 = omega.reshape(N_CORES, P, F)
    xin = np.concatenate([re8, im8, om8], axis=2)  # [8, P, 3F]
    in_maps = [{"x_in": np.ascontiguousarray(xin[i])} for i in range(N_CORES)]
    res = run_bass_kernel_spmd(nc, in_maps, list(range(N_CORES)), trace=_trace)
    re_full = np.concatenate(
        [res.results[i]["x_out"][:, 0:F].reshape(-1) for i in range(N_CORES)]
    )
    im_full = np.concatenate(
        [res.results[i]["x_out"][:, F : 2 * F].reshape(-1) for i in range(N_CORES)]
    )
    if _trace:
        kernel.last_results = res
    return re_full.astype(np.float32), im_full.astype(np.float32)
